# revision 28
# baseline (speedup 1.0000x reference)
"""Trainium2 Bass kernel for CNNLayer: conv(K=3 along H) + bias + tanh + topk(50) along H.

Full input contract:
  x:      [1024, 1, 200, 32] f32
  conv_w: [16, 1, 3, 1]      f32
  conv_b: [16]               f32
Output: [1024, 16, 50, 32] f32 (top-50 along H, sorted descending, after tanh)

Strategy: data-parallel over batch across 8 cores (128 samples/core).
Per sample:
  - load x[s]=[200,32] h-partitioned, PE-transpose (identity matmul) to
    xT[w, h]; 4 small DMAs repack to x8[w8, (wblock, h)] at partition base 0
  - conv z[(o,w8), h] per wblock: 3 accumulating K=8 PE matmuls (one per
    tap) with block-diagonal weights; shifted column ranges handle the
    'same' padding exactly
  - top-k on raw z (tanh is monotonic so topk commutes past bias+tanh):
    7x DVE max (top-8, sorted desc) + 6x match_replace (-1e30 masking)
    per (o,w8)-row over the 200 h values; rounds interleaved across the
    4 wblocks so the DVE pipeline never stalls (~99% DVE occupancy)
  - tanh(top50 + bias[o]) on ACT, fused with the PSUM eviction path
  - PE-transpose result to [r, (o,w8)] so the output DMA is w-contiguous
"""

import os
import sys

for _p in ("/opt/trn_rl_repo", "/root/.axon_site/_ro/trn_rl_repo"):
    if os.path.isdir(_p) and _p not in sys.path:
        sys.path.insert(0, _p)

import numpy as np

N_CORES = 8
B, H, W = 1024, 200, 32
COUT, KH, TOPK = 16, 3, 50
BS = B // N_CORES  # samples per core
NWB = W // 8       # 4 w-blocks of 8 -> (o, w8) = 128 partition rows
NV = 56            # values extracted per row (7 rounds of 8)

_CACHE = {}


def build_module(n_samples=BS, bufs=None, topk_repeat=1, conv_repeat=1, act_repeat=1, topk_dtype=None):
    import concourse.bass as bass  # noqa: F401
    import concourse.tile as tile
    from concourse import bacc, mybir

    _bufs = dict(xin=12, xt=12, xt3=12, zpsum=4, zs=10, v=10, res=10,
                 otpsum=2, xtpsum=2, u=12)
    _bufs.update(bufs or {})
    bufs = _bufs
    f32 = mybir.dt.float32
    tdt = f32 if topk_dtype is None else getattr(mybir.dt, topk_dtype)
    nc = bacc.Bacc("TRN2", target_bir_lowering=False, debug=False,
                   num_devices=N_CORES)

    x = nc.dram_tensor("x", [n_samples, H, W], f32, kind="ExternalInput").ap()
    wkj = nc.dram_tensor("wkj", [KH * 8, 128], f32, kind="ExternalInput").ap()
    bias = nc.dram_tensor("bias_p", [128, 1], f32, kind="ExternalInput").ap()
    ident = nc.dram_tensor("ident", [128, 128], f32, kind="ExternalInput").ap()
    out = nc.dram_tensor("out", [n_samples, COUT, TOPK, W], f32,
                         kind="ExternalOutput").ap()

    with tile.TileContext(nc) as tc:
        with (
            tc.tile_pool(name="const", bufs=1) as constp,
            tc.tile_pool(name="xin", bufs=bufs["xin"]) as xinp,
            tc.tile_pool(name="xt", bufs=bufs["xt"]) as xtp,
            tc.tile_pool(name="xt3", bufs=bufs["xt3"]) as xt3p,
            tc.tile_pool(name="zpsum", bufs=bufs["zpsum"], space="PSUM") as zpsum,
            tc.tile_pool(name="zs", bufs=bufs["zs"]) as zsp,
            tc.tile_pool(name="v", bufs=bufs["v"]) as vp,
            tc.tile_pool(name="res", bufs=bufs["res"]) as resp,
            tc.tile_pool(name="otpsum", bufs=bufs["otpsum"], space="PSUM") as otpsum,
            tc.tile_pool(name="xtpsum", bufs=bufs["xtpsum"], space="PSUM") as xtpsum,
            tc.tile_pool(name="u", bufs=bufs["u"]) as up,
        ):
            wk_sb = []
            for k in range(KH):
                wt = constp.tile([8, 128], f32, tag=f"wk{k}")
                nc.sync.dma_start(wt[:], wkj[8 * k:8 * (k + 1), :])
                wk_sb.append(wt)
            ident_sb = constp.tile([128, 128], f32)
            nc.sync.dma_start(ident_sb[:], ident[:])
            bias_sb = constp.tile([128, 1], f32)
            nc.sync.dma_start(bias_sb[:], bias[:])

            pending = []

            def emit_output(s, vt):
                u_t = up.tile([TOPK, COUT * W], f32, tag="u")
                u_view = u_t[:].rearrange("p (o wb w8) -> p o wb w8",
                                          o=COUT, wb=NWB, w8=8)
                for wb in range(NWB):
                    # tanh(top50 + bias)
                    res = resp.tile([128, TOPK], f32, tag="res")
                    nc.scalar.activation(res[:], vt[wb][:, 0:TOPK],
                                         mybir.ActivationFunctionType.Tanh,
                                         bias=bias_sb[:, 0:1])
                    # transpose [(o,w8), r] -> [r, (o,w8)] for w-contiguous
                    oT = otpsum.tile([TOPK, 128], f32, tag="oT")
                    nc.tensor.transpose(oT[:], res[:], ident_sb[:, :])
                    nc.scalar.copy(
                        u_view[:, :, wb, :],
                        oT[:].rearrange("p (o w8) -> p o w8", o=COUT, w8=8),
                    )
                nc.sync.dma_start(
                    out[s].rearrange("o r w -> r o w"),
                    u_t[:].rearrange("p (o w) -> p o w", o=COUT),
                )

            for s in range(n_samples):
                # load x[s]=[200,32] as [h(100), (half, w)]; PE-transpose
                # both halves into one PSUM tile -> xT[w, xrow]
                xin = xinp.tile([100, 64], f32)
                nc.sync.dma_start(
                    xin[:].rearrange("h (hh w) -> h hh w", hh=2),
                    x[s].rearrange("(hh h) w -> h hh w", hh=2),
                )
                xtps = xtpsum.tile([32, H], f32)
                nc.tensor.transpose(xtps[:, 0:100], xin[:, 0:32],
                                    ident_sb[:100, :100])
                nc.tensor.transpose(xtps[:, 100:200], xin[:, 32:64],
                                    ident_sb[:100, :100])
                xT = xtp.tile([32, H], f32)
                nc.scalar.copy(xT[:], xtps[:])

                # xT8[w8, (wb, j)] = x[j, 8wb+w8]: w-blocks into the free dim
                x8 = xt3p.tile([8, NWB * H], f32)
                for wb in range(NWB):
                    nc.sync.dma_start(x8[:, H * wb:H * (wb + 1)],
                                      xT[8 * wb:8 * wb + 8, :])

                # conv per wb: 3 accumulating matmuls (taps), K=8;
                # per-tap column ranges make the H-boundary exact (no pad)
                zs = zsp.tile([128, NWB * H], tdt)
                for wb in range(NWB):
                    z = zpsum.tile([128, H], f32)
                    xv = x8[:, H * wb:H * (wb + 1)]
                    for _cr in range(conv_repeat):
                        nc.tensor.matmul(z[:, 0:H], wk_sb[1][:],
                                         xv[:, 0:H], start=True, stop=False)
                        nc.tensor.matmul(z[:, 1:H], wk_sb[0][:],
                                         xv[:, 0:H - 1], start=False, stop=False)
                        nc.tensor.matmul(z[:, 0:H - 1], wk_sb[2][:],
                                         xv[:, 1:H], start=False, stop=True)
                    for _ar in range(act_repeat):
                        nc.scalar.copy(zs[:, H * wb:H * (wb + 1)], z[:])

                # top-56 via 7 rounds of max8 + match_replace masking;
                # rounds interleaved across w-blocks so DVE alternates
                # independent chains (hides per-op write-ack latency)
                zsl = [zs[:, H * wb:H * (wb + 1)] for wb in range(NWB)]
                vt = []
                for wb in range(NWB):
                    v = vp.tile([128, NV], tdt, tag=f"v{wb}")
                    nc.vector.max(v[:, 0:8], zsl[wb][:])
                    vt.append(v)
                for rep in range(topk_repeat):
                    for r in range(1, 7):
                        for wb in range(NWB):
                            nc.vector.match_replace(
                                zsl[wb][:], vt[wb][:, 8 * r - 8:8 * r],
                                zsl[wb][:], -1e30)
                            nc.vector.max(vt[wb][:, 8 * r:8 * r + 8],
                                          zsl[wb][:])

                # defer the output stage by one sample: ACT/PE are strict
                # FIFO on HW, so emitting tanh(s) here would queue it ahead
                # of sample s+1's PSUM evictions while it waits on topk(s)
                pending.append((s, vt))
                if len(pending) > 1:
                    emit_output(*pending.pop(0))
            while pending:
                emit_output(*pending.pop(0))

    nc.compile()
    return nc


def _prep_consts(conv_w, conv_b):
    conv_w = np.asarray(conv_w, dtype=np.float32)
    conv_b = np.asarray(conv_b, dtype=np.float32)
    wmat = conv_w[:, 0, :, 0]  # [COUT, KH]
    wkj = np.zeros((KH, 8, 128), dtype=np.float32)
    for k in range(KH):
        for o in range(COUT):
            for w8 in range(8):
                wkj[k, w8, o * 8 + w8] = wmat[o, k]
    bias_p = np.repeat(conv_b, 8).astype(np.float32)[:, None]  # [(o,w8), 1]
    ident = np.eye(128, dtype=np.float32)
    return wkj.reshape(KH * 8, 128), bias_p, ident


def get_compiled(n_samples=BS):
    key = n_samples
    if key not in _CACHE:
        _CACHE[key] = build_module(n_samples)
    return _CACHE[key]


def _make_runner(nc):
    """Build a reusable jitted SPMD executor (jit traced once, reused across
    kernel() calls — run_bass_kernel_spmd re-traces on every call)."""
    import jax
    from jax.sharding import Mesh, PartitionSpec
    from jax.experimental.shard_map import shard_map
    from concourse import mybir
    from concourse.bass2jax import (_bass_exec_p, install_neuronx_cc_hook,
                                    partition_id_tensor)

    install_neuronx_cc_hook()
    in_names, out_names, out_avals, out_shapes = [], [], [], []
    pid = nc.partition_id_tensor.name if nc.partition_id_tensor else None
    for alloc in nc.m.functions[0].allocations:
        if not isinstance(alloc, mybir.MemoryLocationSet):
            continue
        name = alloc.memorylocations[0].name
        if alloc.kind == "ExternalInput":
            if name != pid:
                in_names.append(name)
        elif alloc.kind == "ExternalOutput":
            out_names.append(name)
            shape = tuple(alloc.tensor_shape)
            dtype = mybir.dt.np(alloc.dtype)
            out_avals.append(jax.core.ShapedArray(shape, dtype))
            out_shapes.append((shape, dtype))
    n_params = len(in_names)
    all_in = in_names + out_names + ([pid] if pid else [])

    def _body(*args):
        operands = list(args)
        if pid is not None:
            operands.append(partition_id_tensor())
        return tuple(_bass_exec_p.bind(
            *operands, out_avals=tuple(out_avals), in_names=tuple(all_in),
            out_names=tuple(out_names), lowering_input_output_aliases=(),
            sim_require_finite=True, sim_require_nnan=True, nc=nc))

    devices = jax.devices()[:N_CORES]
    assert len(devices) == N_CORES
    mesh = Mesh(np.asarray(devices), ("core",))
    nio = n_params + len(out_names)
    sharded = jax.jit(
        shard_map(_body, mesh=mesh,
                  in_specs=(PartitionSpec("core"),) * nio,
                  out_specs=(PartitionSpec("core"),) * len(out_names),
                  check_rep=False),
        donate_argnums=tuple(range(n_params, nio)), keep_unused=True)

    import jax.numpy as jnp
    from jax.sharding import NamedSharding
    shd = NamedSharding(mesh, PartitionSpec("core"))
    make_zeros = jax.jit(
        lambda: tuple(jnp.zeros((N_CORES * s[0],) + s[1:], d)
                      for s, d in out_shapes),
        out_shardings=(shd,) * len(out_shapes))

    def run(global_inputs):
        ins = [global_inputs[n] for n in in_names]
        zeros = jax.block_until_ready(make_zeros())
        outs = jax.block_until_ready(sharded(*ins, *zeros))
        return {n: np.asarray(o) for n, o in zip(out_names, outs)}

    return run


def kernel(x, conv_w, conv_b):
    x = np.asarray(x, dtype=np.float32)
    nc = get_compiled(BS)
    wkj, bias_p, ident = _prep_consts(conv_w, conv_b)
    xs = np.ascontiguousarray(x.reshape(B, H, W))  # squeeze CIN=1

    if "runner" not in _CACHE:
        try:
            _CACHE["runner"] = _make_runner(nc)
        except Exception:
            _CACHE["runner"] = None
    runner = _CACHE["runner"]

    if runner is not None:
        global_inputs = {
            "x": xs,
            "wkj": np.concatenate([wkj] * N_CORES, axis=0),
            "bias_p": np.concatenate([bias_p] * N_CORES, axis=0),
            "ident": np.concatenate([ident] * N_CORES, axis=0),
        }
        # the axon terminal occasionally throws a transient
        # NRT_EXEC_UNIT_UNRECOVERABLE; a retry on a fresh executable
        # succeeds, so retry before giving up on the fast path
        for attempt in range(3):
            try:
                out = runner(global_inputs)["out"]
                return out.reshape(B, COUT, TOPK, W)
            except Exception:
                import time as _time
                _time.sleep(2.0 * (attempt + 1))
                try:
                    runner = _make_runner(nc)
                    _CACHE["runner"] = runner
                except Exception:
                    break

    # fallback: stock SPMD path (re-traces jit per call)
    from concourse.bass_utils import run_bass_kernel_spmd
    in_maps = []
    for c in range(N_CORES):
        in_maps.append({
            "x": np.ascontiguousarray(xs[c * BS:(c + 1) * BS]),
            "wkj": wkj,
            "bias_p": bias_p,
            "ident": ident,
        })
    last_err = None
    for attempt in range(3):
        try:
            res = run_bass_kernel_spmd(nc, in_maps, list(range(N_CORES)))
            return np.concatenate(
                [res.results[c]["out"] for c in range(N_CORES)], axis=0)
        except Exception as e:
            last_err = e
            import time as _time
            _time.sleep(2.0 * (attempt + 1))
    raise last_err
